# revision 10
# baseline (speedup 1.0000x reference)
"""Trainium2 Bass kernel for nn_MultiHeadAttention_62766652064333.

Reference computation (per batch b, all 8 "heads" identical):
    Ql = Q @ Wq + bq;  Kl = K @ Wk + bk;  Vl = V @ Wv + bv
    scores = Ql @ Kl.T / sqrt(dm) + mask * (-1e9)
    att = softmax(scores, axis=-1)
    head = att @ Vl
    Y = tile(head, h) @ Wl + bl     == head @ (sum of h row-blocks of Wl) + bl
    att_ws = broadcast att over h

Sharding: data-parallel over batch — one batch per NeuronCore (8 cores).

Device dataflow (per core, "transposed" layouts so the PE contraction dim
is always on SBUF partitions; no on-device transposes needed):
    host supplies QT/KT/VT = X[b].T  (d-major [512, 1024])
    QlT[dout, q] = sum_di Wq[di, dout] * QT[di, q]      (lhsT = Wq blocks)
    KlT likewise;  Vl[k, dout] = sum_di VT[di, k] * Wv[di, dout]
    scoresT[k, q] = sum_d KlT[d, k] * QlT[d, q]
    exT = Exp(scoresT / sqrt(dm) + maskbias[k])         (ACT, mask as bias)
    denomR[*, q] = ones128.T @ exT       (replicated rows, for att store)
    denomQ[q]    = exT.T @ ones_col      (q on partitions, for Y scaling)
    att = exT * recip(denomR)  -> DMA (transposed; host untransposes)
    headT[d, q] = sum_k Vl[k, d] * exT[k, q]            (unnormalized)
    Y[q, :] = (headT.T @ Wlsum)[q, :] * recip(denomQ)[q] + bl

All big matmuls run as float32r (1 cycle/row on the PE for N>=256).
"""

import numpy as np
from contextlib import ExitStack

import concourse.bass as bass
import concourse.mybir as mybir
import concourse.tile as tile
from concourse import bacc
from concourse.bass_utils import run_bass_kernel_spmd

P = 128
DM = 512
H = 8
B = 8
SQ = 1024
SK = 1024
ND = DM // P     # 4 d-tiles of 128
NK = SK // P     # 8 k-tiles
NQ = SQ // P     # 8 q-tiles
NF = 512         # matmul moving free dim (fp32 max)
NH = SQ // NF    # 2 q-halves
F32 = mybir.dt.float32
F32R = mybir.dt.float32r
SM_SCALE = float(1.0 / np.sqrt(np.float32(DM)))


def build_bass():
    nc = bacc.Bacc("TRN2", target_bir_lowering=False, debug=False)
    AF = mybir.ActivationFunctionType

    qt_d = nc.dram_tensor("qt", [DM, SQ], F32R, kind="ExternalInput").ap()
    kt_d = nc.dram_tensor("kt", [DM, SK], F32R, kind="ExternalInput").ap()
    vt_d = nc.dram_tensor("vt", [DM, SK], F32R, kind="ExternalInput").ap()
    wq_d = nc.dram_tensor("wq", [DM, DM], F32R, kind="ExternalInput").ap()
    wk_d = nc.dram_tensor("wk", [DM, DM], F32R, kind="ExternalInput").ap()
    wv_d = nc.dram_tensor("wv", [DM, DM], F32R, kind="ExternalInput").ap()
    wl_d = nc.dram_tensor("wls", [DM, DM], F32R, kind="ExternalInput").ap()
    bq_d = nc.dram_tensor("bq", [P, ND], F32, kind="ExternalInput").ap()
    bk_d = nc.dram_tensor("bk", [P, ND], F32, kind="ExternalInput").ap()
    bv_d = nc.dram_tensor("bvr", [P, DM], F32, kind="ExternalInput").ap()
    bl_d = nc.dram_tensor("blr", [P, DM], F32, kind="ExternalInput").ap()
    mb_d = nc.dram_tensor("mb", [P, NK], F32, kind="ExternalInput").ap()
    ones_d = nc.dram_tensor("ones", [P, P], F32R, kind="ExternalInput").ap()

    att_d = nc.dram_tensor("attT", [SK, SQ], F32, kind="ExternalOutput").ap()
    y_d = nc.dram_tensor("y", [SQ, DM], F32, kind="ExternalOutput").ap()

    with tile.TileContext(nc) as tc, ExitStack() as ctx:
        consts = ctx.enter_context(tc.tile_pool(name="consts", bufs=1))
        wpool = ctx.enter_context(tc.tile_pool(name="wpool", bufs=1))
        bigp = ctx.enter_context(tc.tile_pool(name="bigp", bufs=6))
        exp_p = ctx.enter_context(tc.tile_pool(name="exp_p", bufs=1))
        stage = ctx.enter_context(tc.tile_pool(name="stage", bufs=3))
        pwork = ctx.enter_context(tc.tile_pool(name="pwork", bufs=5, space="PSUM"))
        pden = ctx.enter_context(tc.tile_pool(name="pden", bufs=2, space="PSUM"))
        pdnq = ctx.enter_context(tc.tile_pool(name="pdnq", bufs=1, space="PSUM"))

        # --- tiles ---
        ones128 = consts.tile([P, P], F32R, name="ones128", tag="ones128")
        bq_sb = consts.tile([P, ND], F32, name="bq_sb", tag="bq_sb")
        bk_sb = consts.tile([P, ND], F32, name="bk_sb", tag="bk_sb")
        bv_sb = consts.tile([P, DM], F32, name="bv_sb", tag="bv_sb")
        bl_sb = consts.tile([P, DM], F32, name="bl_sb", tag="bl_sb")
        mb_sb = consts.tile([P, NK], F32, name="mb_sb", tag="mb_sb")

        wq_sb = wpool.tile([P, ND, DM], F32R, name="wq_sb", tag="wq_sb")
        wk_sb = wpool.tile([P, ND, DM], F32R, name="wk_sb", tag="wk_sb")
        wv_sb = wpool.tile([P, ND, DM], F32R, name="wv_sb", tag="wv_sb")
        wl_sb = wpool.tile([P, ND, DM], F32R, name="wl_sb", tag="wl_sb")

        # --- big 16KB-class tiles share one rotating tag (6 slots) ---
        def big16(name):
            return bigp.tile([P, ND, SQ], F32R, name=name, tag="big16")

        qt_sb = big16("qt_sb")
        kt_sb = big16("kt_sb")
        vt_sb = big16("vt_sb")

        # --- input DMAs, ordered by first use; split per d-block so the
        # first projection matmuls start after ~0.75MB instead of ~11MB.
        # sync and scalar issue to distinct HWDGE rings (FIFO per ring).
        wq_r = wq_d.rearrange("(o p) f -> p o f", p=P)
        wk_r = wk_d.rearrange("(o p) f -> p o f", p=P)
        wv_r = wv_d.rearrange("(o p) f -> p o f", p=P)
        wl_r = wl_d.rearrange("(o p) f -> p o f", p=P)
        qt_r = qt_d.rearrange("(o p) q -> p o q", p=P)
        kt_r = kt_d.rearrange("(o p) q -> p o q", p=P)
        vt_r = vt_d.rearrange("(o p) q -> p o q", p=P)

        for di in range(ND):
            nc.sync.dma_start(wq_sb[:, di, :], wq_r[:, di, :])
            nc.sync.dma_start(qt_sb[:, di, :], qt_r[:, di, :])
        nc.sync.dma_start(bq_sb[:], bq_d[:])
        nc.sync.dma_start(bk_sb[:], bk_d[:])
        for di in range(ND):
            nc.scalar.dma_start(wk_sb[:, di, :], wk_r[:, di, :])
            nc.scalar.dma_start(kt_sb[:, di, :], kt_r[:, di, :])
        nc.scalar.dma_start(mb_sb[:], mb_d[:])
        for di in range(ND):
            nc.sync.dma_start(vt_sb[:, di, :], vt_r[:, di, :])
            nc.sync.dma_start(wv_sb[:, di, :], wv_r[:, di, :])
        nc.sync.dma_start(bv_sb[:], bv_d[:])
        nc.scalar.dma_start(ones128[:], ones_d[:])
        for di in range(ND):
            nc.scalar.dma_start(wl_sb[:, di, :], wl_r[:, di, :])
        nc.scalar.dma_start(bl_sb[:], bl_d[:])

        qlT = big16("qlT")
        klT = big16("klT")
        # Vl natural layout [k, dout]: [128, 8, 512]
        vl = bigp.tile([P, NK, DM], F32R, name="vl", tag="big16")

        ex = exp_p.tile([P, NK, SQ], F32R, name="ex", tag="ex")
        rc = consts.tile([P, SQ], F32, name="rc", tag="rc")
        rcq = consts.tile([P, NQ], F32, name="rcq", tag="rcq")

        def ps_tile(name):
            return pwork.tile([P, NF], F32, name=name, tag="ps")

        # --- Phase A: QlT / KlT projections ---
        for w_sb, x_sb, out_sb, b_sb, pname in (
            (wq_sb, qt_sb, qlT, bq_sb, "psql"),
            (wk_sb, kt_sb, klT, bk_sb, "pskl"),
        ):
            for dt in range(ND):
                pss = [ps_tile(f"{pname}_{dt}_{qh}") for qh in range(NH)]
                for di in range(ND):
                    for qh in range(NH):
                        nc.tensor.matmul(
                            pss[qh][:],
                            w_sb[:, di, dt * P:(dt + 1) * P],
                            x_sb[:, di, qh * NF:(qh + 1) * NF],
                            start=(di == 0),
                            stop=(di == ND - 1),
                        )
                for qh in range(NH):
                    nc.scalar.activation(
                        out_sb[:, dt, qh * NF:(qh + 1) * NF],
                        pss[qh][:],
                        AF.Identity,
                        bias=b_sb[:, dt:dt + 1],
                        scale=1.0,
                    )

        # --- Phase A': Vl = VT.T @ Wv + bv (natural [k, dout]) ---
        for kt_i in range(NK):
            ps = ps_tile(f"psvl_{kt_i}")
            for di in range(ND):
                nc.tensor.matmul(
                    ps[:],
                    vt_sb[:, di, kt_i * P:(kt_i + 1) * P],
                    wv_sb[:, di, :],
                    start=(di == 0),
                    stop=(di == ND - 1),
                )
            nc.vector.tensor_add(out=vl[:, kt_i, :], in0=ps[:], in1=bv_sb[:])

        # --- Phase B: scoresT -> exp -> denominators ---
        pd = [
            pden.tile([P, NF], F32, name=f"pd_{qh}", tag="pden") for qh in range(NH)
        ]
        for kt_i in range(NK):
            pss = [ps_tile(f"pssc_{kt_i}_{qh}") for qh in range(NH)]
            for di in range(ND):
                for qh in range(NH):
                    nc.tensor.matmul(
                        pss[qh][:],
                        klT[:, di, kt_i * P:(kt_i + 1) * P],
                        qlT[:, di, qh * NF:(qh + 1) * NF],
                        start=(di == 0),
                        stop=(di == ND - 1),
                    )
            for qh in range(NH):
                qs = slice(qh * NF, (qh + 1) * NF)
                nc.scalar.activation(
                    ex[:, kt_i, qs],
                    pss[qh][:],
                    AF.Exp,
                    bias=mb_sb[:, kt_i:kt_i + 1],
                    scale=SM_SCALE,
                )
                nc.tensor.matmul(
                    pd[qh][:],
                    ones128[:],
                    ex[:, kt_i, qs],
                    start=(kt_i == 0),
                    stop=(kt_i == NK - 1),
                )

        # per-q-partition denominator (for Y scaling). fp32r matmul needs a
        # wider dst pattern, so each qi writes 8 identical columns.
        denq = pdnq.tile([P, NQ, 8], F32, name="denq", tag="denq")
        for qi in range(NQ):
            for kt_i in range(NK):
                nc.tensor.matmul(
                    denq[:, qi, :],
                    ex[:, kt_i, qi * P:(qi + 1) * P],
                    ones128[:, 0:8],
                    start=(kt_i == 0),
                    stop=(kt_i == NK - 1),
                )

        # --- reciprocals of denominators ---
        for qh in range(NH):
            nc.vector.reciprocal(rc[:, qh * NF:(qh + 1) * NF], pd[qh][:])
        nc.vector.reciprocal(rcq[:], denq[:, :, 0])

        # --- normalize att into staging, stream out ---
        for kt_i in range(NK):
            att_st = stage.tile([P, SQ], F32, name=f"att_st_{kt_i}", tag="att_st")
            for qh in range(NH):
                qs = slice(qh * NF, (qh + 1) * NF)
                nc.vector.tensor_mul(
                    out=att_st[:, qs], in0=ex.bitcast(F32)[:, kt_i, qs], in1=rc[:, qs]
                )
            nc.sync.dma_start(att_d[kt_i * P:(kt_i + 1) * P, :], att_st[:])

        # --- Phase C: headT[d, q] = Vl.T @ exT (unnormalized) ---
        hT = big16("hT")
        for dt in range(ND):
            pss = [ps_tile(f"pshd_{dt}_{qh}") for qh in range(NH)]
            for kt_i in range(NK):
                for qh in range(NH):
                    nc.tensor.matmul(
                        pss[qh][:],
                        vl[:, kt_i, dt * P:(dt + 1) * P],
                        ex[:, kt_i, qh * NF:(qh + 1) * NF],
                        start=(kt_i == 0),
                        stop=(kt_i == NK - 1),
                    )
            for qh in range(NH):
                nc.scalar.activation(
                    hT[:, dt, qh * NF:(qh + 1) * NF],
                    pss[qh][:],
                    AF.Copy,
                )

        # --- Phase Y: Y[q, :] = (headT.T @ Wlsum) * rcq[q] + bl ---
        for qi in range(NQ):
            ps = ps_tile(f"psy_{qi}")
            for di in range(ND):
                nc.tensor.matmul(
                    ps[:],
                    hT[:, di, qi * P:(qi + 1) * P],
                    wl_sb[:, di, :],
                    start=(di == 0),
                    stop=(di == ND - 1),
                )
            y_sb = stage.tile([P, DM], F32, name=f"y_sb_{qi}", tag="y_sb")
            nc.vector.scalar_tensor_tensor(
                out=y_sb[:],
                in0=ps[:],
                scalar=rcq[:, qi:qi + 1],
                in1=bl_sb[:],
                op0=mybir.AluOpType.mult,
                op1=mybir.AluOpType.add,
            )
            nc.sync.dma_start(y_d[qi * P:(qi + 1) * P, :], y_sb[:])

    nc.compile()
    return nc


_NC_CACHE = {}


def get_nc():
    if "nc" not in _NC_CACHE:
        _NC_CACHE["nc"] = build_bass()
    return _NC_CACHE["nc"]


def prepare_in_maps(Q, K, V, mask, Wq, bq, Wk, bk, Wv, bv, Wl, bl):
    f = lambda a: np.ascontiguousarray(np.asarray(a, dtype=np.float32))
    Q, K, V = f(Q), f(K), f(V)
    Wq, Wk, Wv, Wl = f(Wq), f(Wk), f(Wv), f(Wl)
    bq, bk, bv, bl = f(bq), f(bk), f(bv), f(bl)
    mask = np.asarray(mask)

    wls = np.ascontiguousarray(
        Wl.reshape(H, DM, DM).sum(axis=0, dtype=np.float64).astype(np.float32)
    )
    bq2 = np.ascontiguousarray(bq.reshape(ND, P).T)       # [128, 4]
    bk2 = np.ascontiguousarray(bk.reshape(ND, P).T)
    bvr = np.ascontiguousarray(np.broadcast_to(bv, (P, DM)))  # replicated rows
    blr = np.ascontiguousarray(np.broadcast_to(bl, (P, DM)))

    in_maps = []
    for b in range(B):
        mb = (mask[b, 0].astype(np.float32) * np.float32(-1e9))
        in_maps.append(
            {
                "qt": np.ascontiguousarray(Q[b].T),
                "kt": np.ascontiguousarray(K[b].T),
                "vt": np.ascontiguousarray(V[b].T),
                "wq": Wq,
                "wk": Wk,
                "wv": Wv,
                "wls": wls,
                "bq": bq2,
                "bk": bk2,
                "bvr": bvr,
                "blr": blr,
                "mb": np.ascontiguousarray(mb.reshape(NK, P).T),  # [128, 8]
                "ones": np.ones((P, P), dtype=np.float32),
            }
        )
    return in_maps


def postprocess(results):
    Y = np.stack([np.asarray(results[b]["y"]) for b in range(B)])
    att = np.stack([np.asarray(results[b]["attT"]).T for b in range(B)])
    att_ws = np.broadcast_to(att[:, None], (B, H, SQ, SK))
    return Y, att_ws


def kernel(Q, K, V, mask, Wq, bq, Wk, bk, Wv, bv, Wl, bl):
    nc = get_nc()
    in_maps = prepare_in_maps(Q, K, V, mask, Wq, bq, Wk, bk, Wv, bv, Wl, bl)
    res = run_bass_kernel_spmd(nc, in_maps, list(range(B)))
    return postprocess(res.results)


# revision 11
# speedup vs baseline: 1.3283x; 1.3283x over previous
"""Trainium2 Bass kernel for nn_MultiHeadAttention_62766652064333.

Reference computation (per batch b, all 8 "heads" identical):
    Ql = Q @ Wq + bq;  Kl = K @ Wk + bk;  Vl = V @ Wv + bv
    scores = Ql @ Kl.T / sqrt(dm) + mask * (-1e9)
    att = softmax(scores, axis=-1)
    head = att @ Vl
    Y = tile(head, h) @ Wl + bl     == head @ (sum of h row-blocks of Wl) + bl
    att_ws = broadcast att over h

Sharding: data-parallel over batch — one batch per NeuronCore (8 cores).

Device dataflow (per core, "transposed" layouts so the PE contraction dim
is always on SBUF partitions; no on-device transposes needed):
    host supplies QT/KT/VT = X[b].T  (d-major [512, 1024])
    QlT[dout, q] = sum_di Wq[di, dout] * QT[di, q]      (lhsT = Wq blocks)
    KlT likewise;  Vl[k, dout] = sum_di VT[di, k] * Wv[di, dout]
    scoresT[k, q] = sum_d KlT[d, k] * QlT[d, q]
    exT = Exp(scoresT / sqrt(dm) + maskbias[k])         (ACT, mask as bias)
    denomR[*, q] = ones128.T @ exT       (replicated rows, for att store)
    denomQ[q]    = exT.T @ ones cols     (q on partitions, for Y scaling)
    att = exT * recip(denomR)  -> DMA (transposed; host untransposes)
    headT[d, q] = sum_k Vl[k, d] * exT[k, q]            (unnormalized)
    Y[q, :] = (headT.T @ Wlsum)[q, :] * recip(denomQ)[q] + bl

MM_DT selects the tensor-engine operand dtype: bfloat16 (fast weight
load, half DMA) or float32r (fp32 bits, ~1e-4 accuracy).
"""

import numpy as np
import ml_dtypes
from contextlib import ExitStack

import concourse.bass as bass
import concourse.mybir as mybir
import concourse.tile as tile
from concourse import bacc
from concourse.bass_utils import run_bass_kernel_spmd

P = 128
DM = 512
H = 8
B = 8
SQ = 1024
SK = 1024
ND = DM // P     # 4 d-tiles of 128
NK = SK // P     # 8 k-tiles
NQ = SQ // P     # 8 q-tiles
NF = 512         # matmul moving free dim (one PSUM bank)
NH = SQ // NF    # 2 q-halves
F32 = mybir.dt.float32
F32R = mybir.dt.float32r
BF16 = mybir.dt.bfloat16
SM_SCALE = float(1.0 / np.sqrt(np.float32(DM)))

MM_DT = BF16  # tensor-engine operand dtype: BF16 or F32R


def build_bass(mm_dt=None):
    mm_dt = MM_DT if mm_dt is None else mm_dt
    nc = bacc.Bacc("TRN2", target_bir_lowering=False, debug=False)
    AF = mybir.ActivationFunctionType

    qt_d = nc.dram_tensor("qt", [DM, SQ], mm_dt, kind="ExternalInput").ap()
    kt_d = nc.dram_tensor("kt", [DM, SK], mm_dt, kind="ExternalInput").ap()
    vt_d = nc.dram_tensor("vt", [DM, SK], mm_dt, kind="ExternalInput").ap()
    wq_d = nc.dram_tensor("wq", [DM, DM], mm_dt, kind="ExternalInput").ap()
    wk_d = nc.dram_tensor("wk", [DM, DM], mm_dt, kind="ExternalInput").ap()
    wv_d = nc.dram_tensor("wv", [DM, DM], mm_dt, kind="ExternalInput").ap()
    wl_d = nc.dram_tensor("wls", [DM, DM], mm_dt, kind="ExternalInput").ap()
    bq_d = nc.dram_tensor("bq", [P, ND], F32, kind="ExternalInput").ap()
    bk_d = nc.dram_tensor("bk", [P, ND], F32, kind="ExternalInput").ap()
    bv_d = nc.dram_tensor("bvr", [P, DM], F32, kind="ExternalInput").ap()
    bl_d = nc.dram_tensor("blr", [P, DM], F32, kind="ExternalInput").ap()
    mb_d = nc.dram_tensor("mb", [P, NK], F32, kind="ExternalInput").ap()
    ones_d = nc.dram_tensor("ones", [P, P], mm_dt, kind="ExternalInput").ap()

    att_d = nc.dram_tensor("attT", [SK, SQ], F32, kind="ExternalOutput").ap()
    y_d = nc.dram_tensor("y", [SQ, DM], F32, kind="ExternalOutput").ap()

    with tile.TileContext(nc) as tc, ExitStack() as ctx:
        consts = ctx.enter_context(tc.tile_pool(name="consts", bufs=1))
        wpool = ctx.enter_context(tc.tile_pool(name="wpool", bufs=1))
        bigp = ctx.enter_context(tc.tile_pool(name="bigp", bufs=6))
        exp_p = ctx.enter_context(tc.tile_pool(name="exp_p", bufs=1))
        stage = ctx.enter_context(tc.tile_pool(name="stage", bufs=3))
        pwork = ctx.enter_context(tc.tile_pool(name="pwork", bufs=5, space="PSUM"))
        pden = ctx.enter_context(tc.tile_pool(name="pden", bufs=2, space="PSUM"))
        pdnq = ctx.enter_context(tc.tile_pool(name="pdnq", bufs=1, space="PSUM"))

        # --- tiles ---
        ones128 = consts.tile([P, P], mm_dt, name="ones128", tag="ones128")
        bq_sb = consts.tile([P, ND], F32, name="bq_sb", tag="bq_sb")
        bk_sb = consts.tile([P, ND], F32, name="bk_sb", tag="bk_sb")
        bv_sb = consts.tile([P, DM], F32, name="bv_sb", tag="bv_sb")
        bl_sb = consts.tile([P, DM], F32, name="bl_sb", tag="bl_sb")
        mb_sb = consts.tile([P, NK], F32, name="mb_sb", tag="mb_sb")

        wq_sb = wpool.tile([P, ND, DM], mm_dt, name="wq_sb", tag="wq_sb")
        wk_sb = wpool.tile([P, ND, DM], mm_dt, name="wk_sb", tag="wk_sb")
        wv_sb = wpool.tile([P, ND, DM], mm_dt, name="wv_sb", tag="wv_sb")
        wl_sb = wpool.tile([P, ND, DM], mm_dt, name="wl_sb", tag="wl_sb")

        # --- big tiles share one rotating tag (6 slots) ---
        def big16(name):
            return bigp.tile([P, ND, SQ], mm_dt, name=name, tag="big16")

        qt_sb = big16("qt_sb")
        kt_sb = big16("kt_sb")
        vt_sb = big16("vt_sb")

        # --- input DMAs, ordered by first use; split per d-block so the
        # first projection matmuls start early. sync and scalar issue to
        # distinct HWDGE rings (FIFO per ring).
        wq_r = wq_d.rearrange("(o p) f -> p o f", p=P)
        wk_r = wk_d.rearrange("(o p) f -> p o f", p=P)
        wv_r = wv_d.rearrange("(o p) f -> p o f", p=P)
        wl_r = wl_d.rearrange("(o p) f -> p o f", p=P)
        qt_r = qt_d.rearrange("(o p) q -> p o q", p=P)
        kt_r = kt_d.rearrange("(o p) q -> p o q", p=P)
        vt_r = vt_d.rearrange("(o p) q -> p o q", p=P)

        for di in range(ND):
            nc.sync.dma_start(wq_sb[:, di, :], wq_r[:, di, :])
            nc.sync.dma_start(qt_sb[:, di, :], qt_r[:, di, :])
        nc.sync.dma_start(bq_sb[:], bq_d[:])
        nc.sync.dma_start(bk_sb[:], bk_d[:])
        for di in range(ND):
            nc.scalar.dma_start(wk_sb[:, di, :], wk_r[:, di, :])
            nc.scalar.dma_start(kt_sb[:, di, :], kt_r[:, di, :])
        nc.scalar.dma_start(mb_sb[:], mb_d[:])
        for di in range(ND):
            nc.sync.dma_start(vt_sb[:, di, :], vt_r[:, di, :])
            nc.sync.dma_start(wv_sb[:, di, :], wv_r[:, di, :])
        nc.sync.dma_start(bv_sb[:], bv_d[:])
        nc.scalar.dma_start(ones128[:], ones_d[:])
        for di in range(ND):
            nc.scalar.dma_start(wl_sb[:, di, :], wl_r[:, di, :])
        nc.scalar.dma_start(bl_sb[:], bl_d[:])

        qlT = big16("qlT")
        klT = big16("klT")
        # Vl natural layout [k, dout]: [128, 8, 512]
        vl = bigp.tile([P, NK, DM], mm_dt, name="vl", tag="big16")

        ex = exp_p.tile([P, NK, SQ], mm_dt, name="ex", tag="ex")
        rc = consts.tile([P, SQ], F32, name="rc", tag="rc")
        rcq = consts.tile([P, NQ], F32, name="rcq", tag="rcq")

        def ps_tile(name):
            return pwork.tile([P, NF], F32, name=name, tag="ps")

        # --- Phase A: QlT / KlT projections ---
        for w_sb, x_sb, out_sb, b_sb, pname in (
            (wq_sb, qt_sb, qlT, bq_sb, "psql"),
            (wk_sb, kt_sb, klT, bk_sb, "pskl"),
        ):
            for dt in range(ND):
                pss = [ps_tile(f"{pname}_{dt}_{qh}") for qh in range(NH)]
                for di in range(ND):
                    for qh in range(NH):
                        nc.tensor.matmul(
                            pss[qh][:],
                            w_sb[:, di, dt * P:(dt + 1) * P],
                            x_sb[:, di, qh * NF:(qh + 1) * NF],
                            start=(di == 0),
                            stop=(di == ND - 1),
                        )
                for qh in range(NH):
                    nc.scalar.activation(
                        out_sb[:, dt, qh * NF:(qh + 1) * NF],
                        pss[qh][:],
                        AF.Identity,
                        bias=b_sb[:, dt:dt + 1],
                        scale=1.0,
                    )

        # --- Phase A': Vl = VT.T @ Wv + bv (natural [k, dout]) ---
        for kt_i in range(NK):
            ps = ps_tile(f"psvl_{kt_i}")
            for di in range(ND):
                nc.tensor.matmul(
                    ps[:],
                    vt_sb[:, di, kt_i * P:(kt_i + 1) * P],
                    wv_sb[:, di, :],
                    start=(di == 0),
                    stop=(di == ND - 1),
                )
            nc.vector.tensor_add(out=vl[:, kt_i, :], in0=ps[:], in1=bv_sb[:])

        # --- Phase B: scoresT -> exp -> denominators ---
        pd = [
            pden.tile([P, NF], F32, name=f"pd_{qh}", tag="pden") for qh in range(NH)
        ]
        for kt_i in range(NK):
            pss = [ps_tile(f"pssc_{kt_i}_{qh}") for qh in range(NH)]
            for di in range(ND):
                for qh in range(NH):
                    nc.tensor.matmul(
                        pss[qh][:],
                        klT[:, di, kt_i * P:(kt_i + 1) * P],
                        qlT[:, di, qh * NF:(qh + 1) * NF],
                        start=(di == 0),
                        stop=(di == ND - 1),
                    )
            for qh in range(NH):
                qs = slice(qh * NF, (qh + 1) * NF)
                nc.scalar.activation(
                    ex[:, kt_i, qs],
                    pss[qh][:],
                    AF.Exp,
                    bias=mb_sb[:, kt_i:kt_i + 1],
                    scale=SM_SCALE,
                )
                nc.tensor.matmul(
                    pd[qh][:],
                    ones128[:],
                    ex[:, kt_i, qs],
                    start=(kt_i == 0),
                    stop=(kt_i == NK - 1),
                )

        # per-q-partition denominator (for Y scaling); 8 identical columns
        # per qi (fp32r matmul dst-pattern constraint; harmless for bf16).
        denq = pdnq.tile([P, NQ, 8], F32, name="denq", tag="denq")
        for qi in range(NQ):
            for kt_i in range(NK):
                nc.tensor.matmul(
                    denq[:, qi, :],
                    ex[:, kt_i, qi * P:(qi + 1) * P],
                    ones128[:, 0:8],
                    start=(kt_i == 0),
                    stop=(kt_i == NK - 1),
                )

        # --- reciprocals of denominators ---
        for qh in range(NH):
            nc.vector.reciprocal(rc[:, qh * NF:(qh + 1) * NF], pd[qh][:])
        nc.vector.reciprocal(rcq[:], denq[:, :, 0])

        # --- normalize att into staging, stream out ---
        exf = ex.bitcast(F32) if mm_dt == F32R else ex
        for kt_i in range(NK):
            att_st = stage.tile([P, SQ], F32, name=f"att_st_{kt_i}", tag="att_st")
            for qh in range(NH):
                qs = slice(qh * NF, (qh + 1) * NF)
                nc.vector.tensor_mul(
                    out=att_st[:, qs], in0=exf[:, kt_i, qs], in1=rc[:, qs]
                )
            nc.sync.dma_start(att_d[kt_i * P:(kt_i + 1) * P, :], att_st[:])

        # --- Phase C: headT[d, q] = Vl.T @ exT (unnormalized) ---
        hT = big16("hT")
        for dt in range(ND):
            pss = [ps_tile(f"pshd_{dt}_{qh}") for qh in range(NH)]
            for kt_i in range(NK):
                for qh in range(NH):
                    nc.tensor.matmul(
                        pss[qh][:],
                        vl[:, kt_i, dt * P:(dt + 1) * P],
                        ex[:, kt_i, qh * NF:(qh + 1) * NF],
                        start=(kt_i == 0),
                        stop=(kt_i == NK - 1),
                    )
            for qh in range(NH):
                nc.scalar.activation(
                    hT[:, dt, qh * NF:(qh + 1) * NF],
                    pss[qh][:],
                    AF.Copy,
                )

        # --- Phase Y: Y[q, :] = (headT.T @ Wlsum) * rcq[q] + bl ---
        for qi in range(NQ):
            ps = ps_tile(f"psy_{qi}")
            for di in range(ND):
                nc.tensor.matmul(
                    ps[:],
                    hT[:, di, qi * P:(qi + 1) * P],
                    wl_sb[:, di, :],
                    start=(di == 0),
                    stop=(di == ND - 1),
                )
            y_sb = stage.tile([P, DM], F32, name=f"y_sb_{qi}", tag="y_sb")
            nc.vector.scalar_tensor_tensor(
                out=y_sb[:],
                in0=ps[:],
                scalar=rcq[:, qi:qi + 1],
                in1=bl_sb[:],
                op0=mybir.AluOpType.mult,
                op1=mybir.AluOpType.add,
            )
            nc.sync.dma_start(y_d[qi * P:(qi + 1) * P, :], y_sb[:])

    nc.compile()
    return nc


_NC_CACHE = {}


def get_nc():
    if "nc" not in _NC_CACHE:
        _NC_CACHE["nc"] = build_bass()
    return _NC_CACHE["nc"]


def prepare_in_maps(Q, K, V, mask, Wq, bq, Wk, bk, Wv, bv, Wl, bl):
    f = lambda a: np.ascontiguousarray(np.asarray(a, dtype=np.float32))
    Q, K, V = f(Q), f(K), f(V)
    Wq, Wk, Wv, Wl = f(Wq), f(Wk), f(Wv), f(Wl)
    bq, bk, bv, bl = f(bq), f(bk), f(bv), f(bl)
    mask = np.asarray(mask)

    mm_np = ml_dtypes.bfloat16 if MM_DT == BF16 else np.float32
    g = lambda a: np.ascontiguousarray(a.astype(mm_np))

    wls = Wl.reshape(H, DM, DM).sum(axis=0, dtype=np.float64).astype(np.float32)
    bq2 = np.ascontiguousarray(bq.reshape(ND, P).T)       # [128, 4]
    bk2 = np.ascontiguousarray(bk.reshape(ND, P).T)
    bvr = np.ascontiguousarray(np.broadcast_to(bv, (P, DM)))  # replicated rows
    blr = np.ascontiguousarray(np.broadcast_to(bl, (P, DM)))

    in_maps = []
    for b in range(B):
        mb = (mask[b, 0].astype(np.float32) * np.float32(-1e9))
        in_maps.append(
            {
                "qt": g(Q[b].T),
                "kt": g(K[b].T),
                "vt": g(V[b].T),
                "wq": g(Wq),
                "wk": g(Wk),
                "wv": g(Wv),
                "wls": g(wls),
                "bq": bq2,
                "bk": bk2,
                "bvr": bvr,
                "blr": blr,
                "mb": np.ascontiguousarray(mb.reshape(NK, P).T),  # [128, 8]
                "ones": np.ones((P, P), dtype=mm_np),
            }
        )
    return in_maps


def postprocess(results):
    Y = np.stack([np.asarray(results[b]["y"]) for b in range(B)])
    att = np.stack([np.asarray(results[b]["attT"]).T for b in range(B)])
    att_ws = np.broadcast_to(att[:, None], (B, H, SQ, SK))
    return Y, att_ws


def kernel(Q, K, V, mask, Wq, bq, Wk, bk, Wv, bv, Wl, bl):
    nc = get_nc()
    in_maps = prepare_in_maps(Q, K, V, mask, Wq, bq, Wk, bk, Wv, bv, Wl, bl)
    res = run_bass_kernel_spmd(nc, in_maps, list(range(B)))
    return postprocess(res.results)


# revision 13
# speedup vs baseline: 1.5616x; 1.1756x over previous
"""Trainium2 Bass kernel for nn_MultiHeadAttention_62766652064333.

Reference computation (per batch b, all 8 "heads" identical):
    Ql = Q @ Wq + bq;  Kl = K @ Wk + bk;  Vl = V @ Wv + bv
    scores = Ql @ Kl.T / sqrt(dm) + mask * (-1e9)
    att = softmax(scores, axis=-1)
    head = att @ Vl
    Y = tile(head, h) @ Wl + bl     == head @ Wlsum + bl   (identical heads)
    att_ws = broadcast att over h

Algebraic restructuring (host does weight-only preprocessing):
    M    = Wq @ Wk.T                so  Ql @ Kl.T = Q @ M @ K.T + rank-1 terms
    WVL  = Wv @ Wlsum               so  head @ Wlsum = att @ V @ WVL + bv-term
    u[k] = K @ (Wk @ bq)            the only bias term that survives softmax
                                    (bk- and const-terms are per-row constants,
                                     softmax is invariant to them)
    bl2  = bv @ Wlsum + bl          (rows of att sum to 1)

Sharding: data-parallel over batch — one batch per NeuronCore (8 cores).

Device dataflow (per core; PE contraction dim always on SBUF partitions,
no on-device transposes — host supplies QT/KT/VT = X[b].T):
    AT[do, q]   = sum_di M[di, do] QT[di, q]          32 MM
    Vl2[k, do]  = sum_di VT[di, k] WVL[di, do]        32 MM
    scoresT[k,q]= sum_do KT[do, k] AT[do, q]          64 MM
    exT         = Exp(scoresT/sqrt(dm) + mb[k])       ACT (mask+u bias)
    denom       = ones128.T @ exT                     16 MM (replicated rows)
    att         = exT * recip(denom)   -> f32 DMA (transposed; host undoes)
                                       -> bf16 att_n for the Y matmuls
    Y[q, :]     = sum_kt att_n[kt,q-block].T @ Vl2[kt] + bl2   64 MM

All tensor-engine operands are bfloat16 (FWL weight loads fully hidden).
"""

import numpy as np
import ml_dtypes
from contextlib import ExitStack

import concourse.bass as bass
import concourse.mybir as mybir
import concourse.tile as tile
from concourse import bacc
from concourse.bass_utils import run_bass_kernel_spmd

P = 128
DM = 512
H = 8
B = 8
SQ = 1024
SK = 1024
ND = DM // P     # 4 d-tiles of 128
NK = SK // P     # 8 k-tiles
NQ = SQ // P     # 8 q-tiles
NF = 512         # matmul moving free dim (one PSUM bank)
NH = SQ // NF    # 2 q-halves
F32 = mybir.dt.float32
BF16 = mybir.dt.bfloat16
SM_SCALE = float(1.0 / np.sqrt(np.float32(DM)))


def build_bass():
    nc = bacc.Bacc("TRN2", target_bir_lowering=False, debug=False)
    AF = mybir.ActivationFunctionType

    qt_d = nc.dram_tensor("qt", [DM, SQ], BF16, kind="ExternalInput").ap()
    kt_d = nc.dram_tensor("kt", [DM, SK], BF16, kind="ExternalInput").ap()
    vt_d = nc.dram_tensor("vt", [DM, SK], BF16, kind="ExternalInput").ap()
    m_d = nc.dram_tensor("m", [DM, DM], BF16, kind="ExternalInput").ap()
    wvl_d = nc.dram_tensor("wvl", [DM, DM], BF16, kind="ExternalInput").ap()
    bl_d = nc.dram_tensor("blr2", [P, DM], F32, kind="ExternalInput").ap()
    mb_d = nc.dram_tensor("mb", [P, NK], F32, kind="ExternalInput").ap()
    ones_d = nc.dram_tensor("ones", [P, P], BF16, kind="ExternalInput").ap()

    att_d = nc.dram_tensor("attT", [SK, SQ], F32, kind="ExternalOutput").ap()
    y_d = nc.dram_tensor("y", [SQ, DM], F32, kind="ExternalOutput").ap()

    with tile.TileContext(nc) as tc, ExitStack() as ctx:
        consts = ctx.enter_context(tc.tile_pool(name="consts", bufs=1))
        bigp = ctx.enter_context(tc.tile_pool(name="bigp", bufs=1))
        stage = ctx.enter_context(tc.tile_pool(name="stage", bufs=3))
        pwork = ctx.enter_context(tc.tile_pool(name="pwork", bufs=5, space="PSUM"))
        pden = ctx.enter_context(tc.tile_pool(name="pden", bufs=2, space="PSUM"))

        # --- tiles ---
        ones128 = consts.tile([P, P], BF16, name="ones128", tag="ones128")
        bl_sb = consts.tile([P, DM], F32, name="bl_sb", tag="bl_sb")
        mb_sb = consts.tile([P, NK], F32, name="mb_sb", tag="mb_sb")
        m_sb = consts.tile([P, ND, DM], BF16, name="m_sb", tag="m_sb")
        wvl_sb = consts.tile([P, ND, DM], BF16, name="wvl_sb", tag="wvl_sb")

        qt_sb = bigp.tile([P, ND, SQ], BF16, name="qt_sb", tag="qt_sb")
        kt_sb = bigp.tile([P, ND, SK], BF16, name="kt_sb", tag="kt_sb")
        vt_sb = bigp.tile([P, ND, SK], BF16, name="vt_sb", tag="vt_sb")
        at_sb = bigp.tile([P, ND, SQ], BF16, name="at_sb", tag="at_sb")
        vl = bigp.tile([P, NK, DM], BF16, name="vl", tag="vl")
        ex = bigp.tile([P, NK, SQ], BF16, name="ex", tag="ex")
        att_n = bigp.tile([P, NK, SQ], BF16, name="att_n", tag="att_n")
        rc = consts.tile([P, SQ], F32, name="rc", tag="rc")

        # --- input DMAs, ordered by first use, split per d-block; sync and
        # scalar issue to distinct HWDGE rings (FIFO per ring).
        m_r = m_d.rearrange("(o p) f -> p o f", p=P)
        wvl_r = wvl_d.rearrange("(o p) f -> p o f", p=P)
        qt_r = qt_d.rearrange("(o p) q -> p o q", p=P)
        kt_r = kt_d.rearrange("(o p) q -> p o q", p=P)
        vt_r = vt_d.rearrange("(o p) q -> p o q", p=P)

        for di in range(ND):
            nc.sync.dma_start(m_sb[:, di, :], m_r[:, di, :])
            nc.sync.dma_start(qt_sb[:, di, :], qt_r[:, di, :])
        for di in range(ND):
            nc.scalar.dma_start(kt_sb[:, di, :], kt_r[:, di, :])
        nc.scalar.dma_start(mb_sb[:], mb_d[:])
        nc.scalar.dma_start(ones128[:], ones_d[:])
        for di in range(ND):
            nc.sync.dma_start(vt_sb[:, di, :], vt_r[:, di, :])
            nc.sync.dma_start(wvl_sb[:, di, :], wvl_r[:, di, :])
        nc.scalar.dma_start(bl_sb[:], bl_d[:])

        def ps_tile(name):
            return pwork.tile([P, NF], F32, name=name, tag="ps")

        # --- Phase A: AT = M.T @ QT ---
        for dt in range(ND):
            pss = [ps_tile(f"psat_{dt}_{qh}") for qh in range(NH)]
            for di in range(ND):
                for qh in range(NH):
                    nc.tensor.matmul(
                        pss[qh][:],
                        m_sb[:, di, dt * P:(dt + 1) * P],
                        qt_sb[:, di, qh * NF:(qh + 1) * NF],
                        start=(di == 0),
                        stop=(di == ND - 1),
                    )
            for qh in range(NH):
                nc.scalar.activation(
                    at_sb[:, dt, qh * NF:(qh + 1) * NF], pss[qh][:], AF.Copy
                )

        # --- Phase B: scoresT -> exp -> denominator ---
        pd = [
            pden.tile([P, NF], F32, name=f"pd_{qh}", tag="pden") for qh in range(NH)
        ]
        for kt_i in range(NK):
            pss = [ps_tile(f"pssc_{kt_i}_{qh}") for qh in range(NH)]
            for di in range(ND):
                for qh in range(NH):
                    nc.tensor.matmul(
                        pss[qh][:],
                        kt_sb[:, di, kt_i * P:(kt_i + 1) * P],
                        at_sb[:, di, qh * NF:(qh + 1) * NF],
                        start=(di == 0),
                        stop=(di == ND - 1),
                    )
            for qh in range(NH):
                qs = slice(qh * NF, (qh + 1) * NF)
                nc.scalar.activation(
                    ex[:, kt_i, qs],
                    pss[qh][:],
                    AF.Exp,
                    bias=mb_sb[:, kt_i:kt_i + 1],
                    scale=SM_SCALE,
                )
                nc.tensor.matmul(
                    pd[qh][:],
                    ones128[:],
                    ex[:, kt_i, qs],
                    start=(kt_i == 0),
                    stop=(kt_i == NK - 1),
                )

        # --- Phase A' (placed here so the PE fills the recip bubble):
        # Vl2 = VT.T @ WVL ---
        for kt_i in range(NK):
            ps = ps_tile(f"psvl_{kt_i}")
            for di in range(ND):
                nc.tensor.matmul(
                    ps[:],
                    vt_sb[:, di, kt_i * P:(kt_i + 1) * P],
                    wvl_sb[:, di, :],
                    start=(di == 0),
                    stop=(di == ND - 1),
                )
            nc.scalar.activation(vl[:, kt_i, :], ps[:], AF.Copy)

        # --- reciprocal of denominator (replicated rows) ---
        for qh in range(NH):
            nc.vector.reciprocal(rc[:, qh * NF:(qh + 1) * NF], pd[qh][:])

        # --- normalize att: f32 staging for DMA out + bf16 copy for Y ---
        for kt_i in range(NK):
            att_st = stage.tile([P, SQ], F32, name=f"att_st_{kt_i}", tag="att_st")
            for qh in range(NH):
                qs = slice(qh * NF, (qh + 1) * NF)
                nc.vector.tensor_mul(
                    out=att_n[:, kt_i, qs], in0=ex[:, kt_i, qs], in1=rc[:, qs]
                )
                nc.vector.tensor_mul(
                    out=att_st[:, qs], in0=ex[:, kt_i, qs], in1=rc[:, qs]
                )
            nc.sync.dma_start(att_d[kt_i * P:(kt_i + 1) * P, :], att_st[:])

        # --- Phase Y: Y[q, :] = sum_kt att_n[kt].T @ Vl2[kt] + bl2 ---
        for qi in range(NQ):
            ps = ps_tile(f"psy_{qi}")
            for kt_i in range(NK):
                nc.tensor.matmul(
                    ps[:],
                    att_n[:, kt_i, qi * P:(qi + 1) * P],
                    vl[:, kt_i, :],
                    start=(kt_i == 0),
                    stop=(kt_i == NK - 1),
                )
            y_sb = stage.tile([P, DM], F32, name=f"y_sb_{qi}", tag="y_sb")
            nc.vector.tensor_add(out=y_sb[:], in0=ps[:], in1=bl_sb[:])
            nc.sync.dma_start(y_d[qi * P:(qi + 1) * P, :], y_sb[:])

    nc.compile()
    return nc


_NC_CACHE = {}


def get_nc():
    if "nc" not in _NC_CACHE:
        _NC_CACHE["nc"] = build_bass()
    return _NC_CACHE["nc"]


def prepare_in_maps(Q, K, V, mask, Wq, bq, Wk, bk, Wv, bv, Wl, bl):
    f = lambda a: np.ascontiguousarray(np.asarray(a, dtype=np.float32))
    Q, K, V = f(Q), f(K), f(V)
    Wq, Wk, Wv, Wl = f(Wq), f(Wk), f(Wv), f(Wl)
    bq, bk, bv, bl = f(bq), f(bk), f(bv), f(bl)
    mask = np.asarray(mask)

    bf = ml_dtypes.bfloat16
    g = lambda a: np.ascontiguousarray(a.astype(bf))

    wls = Wl.reshape(H, DM, DM).sum(axis=0, dtype=np.float64)
    m = (Wq.astype(np.float64) @ Wk.astype(np.float64).T).astype(np.float32)
    wvl = (Wv.astype(np.float64) @ wls).astype(np.float32)
    bl2 = (bv.astype(np.float64) @ wls + bl).astype(np.float32)
    blr2 = np.ascontiguousarray(np.broadcast_to(bl2, (P, DM)))
    wkbq = Wk @ bq  # [512]; u = K @ wkbq is the only surviving bias term

    in_maps = []
    for b in range(B):
        u = K[b] @ wkbq                                   # [1024]
        mb = mask[b, 0].astype(np.float32) * np.float32(-1e9) \
            + np.float32(SM_SCALE) * u
        in_maps.append(
            {
                "qt": g(Q[b].T),
                "kt": g(K[b].T),
                "vt": g(V[b].T),
                "m": g(m),
                "wvl": g(wvl),
                "blr2": blr2,
                "mb": np.ascontiguousarray(mb.reshape(NK, P).T),  # [128, 8]
                "ones": np.ones((P, P), dtype=bf),
            }
        )
    return in_maps


def postprocess(results):
    Y = np.stack([np.asarray(results[b]["y"]) for b in range(B)])
    att = np.stack([np.asarray(results[b]["attT"]).T for b in range(B)])
    att_ws = np.broadcast_to(att[:, None], (B, H, SQ, SK))
    return Y, att_ws


def kernel(Q, K, V, mask, Wq, bq, Wk, bk, Wv, bv, Wl, bl):
    nc = get_nc()
    in_maps = prepare_in_maps(Q, K, V, mask, Wq, bq, Wk, bk, Wv, bv, Wl, bl)
    res = run_bass_kernel_spmd(nc, in_maps, list(range(B)))
    return postprocess(res.results)


# revision 18
# speedup vs baseline: 1.7117x; 1.0961x over previous
"""Trainium2 Bass kernel for nn_MultiHeadAttention_62766652064333.

Reference computation (per batch b, all 8 "heads" identical):
    Ql = Q @ Wq + bq;  Kl = K @ Wk + bk;  Vl = V @ Wv + bv
    scores = Ql @ Kl.T / sqrt(dm) + mask * (-1e9)
    att = softmax(scores, axis=-1)
    head = att @ Vl
    Y = tile(head, h) @ Wl + bl     == head @ Wlsum + bl   (identical heads)
    att_ws = broadcast att over h

Algebraic restructuring (host does weight-only preprocessing):
    M    = Wq @ Wk.T                so  Ql @ Kl.T = Q @ M @ K.T + rank-1 terms
    WVL  = Wv @ Wlsum               so  head @ Wlsum = att @ V @ WVL + bv-term
    u[k] = K @ (Wk @ bq)            the only bias term that survives softmax
                                    (bk- and const-terms are per-row constants,
                                     softmax is invariant to them)
    bl2  = bv @ Wlsum + bl          (rows of att sum to 1)

Sharding: data-parallel over batch — one batch per NeuronCore (8 cores).

Device dataflow (per core; PE contraction dim always on SBUF partitions,
no on-device transposes — host supplies QT/KT/VT = X[b].T):
    AT[do, q]   = sum_di M[di, do] QT[di, q]          32 MM
    Vl2[k, do]  = sum_di VT[di, k] WVL[di, do]        32 MM
    scoresT[k,q]= sum_do KT[do, k] AT[do, q]          64 MM
    exT         = Exp(scoresT/sqrt(dm) + mb[k])       ACT (mask+u bias)
    denom       = ones128.T @ exT                     16 MM (replicated rows)
    att         = exT * recip(denom)   -> f32 DMA (transposed; host undoes)
                                       -> bf16 att_n for the Y matmuls
    Y[q, :]     = sum_kt att_n[kt,q-block].T @ Vl2[kt] + bl2   64 MM

All tensor-engine operands are bfloat16 (FWL weight loads fully hidden).
"""

import numpy as np
import ml_dtypes
from contextlib import ExitStack

import concourse.bass as bass
import concourse.mybir as mybir
import concourse.tile as tile
from concourse import bacc
from concourse.bass_utils import run_bass_kernel_spmd

P = 128
DM = 512
H = 8
B = 8
SQ = 1024
SK = 1024
ND = DM // P     # 4 d-tiles of 128
NK = SK // P     # 8 k-tiles
NQ = SQ // P     # 8 q-tiles
NF = 512         # matmul moving free dim (one PSUM bank)
NH = SQ // NF    # 2 q-halves
F32 = mybir.dt.float32
BF16 = mybir.dt.bfloat16
SM_SCALE = float(1.0 / np.sqrt(np.float32(DM)))


def build_bass():
    nc = bacc.Bacc("TRN2", target_bir_lowering=False, debug=False)
    AF = mybir.ActivationFunctionType

    qt_d = nc.dram_tensor("qt", [DM, SQ], BF16, kind="ExternalInput").ap()
    kt_d = nc.dram_tensor("kt", [DM, SK], BF16, kind="ExternalInput").ap()
    vt_d = nc.dram_tensor("vt", [DM, SK], BF16, kind="ExternalInput").ap()
    m_d = nc.dram_tensor("m", [DM, DM], BF16, kind="ExternalInput").ap()
    wvl_d = nc.dram_tensor("wvl", [DM, DM], BF16, kind="ExternalInput").ap()
    bl_d = nc.dram_tensor("blr2", [P, DM], F32, kind="ExternalInput").ap()
    mb_d = nc.dram_tensor("mb", [P, NK], F32, kind="ExternalInput").ap()
    ones_d = nc.dram_tensor("ones", [P, P], BF16, kind="ExternalInput").ap()

    att_d = nc.dram_tensor("attT", [SK, SQ], F32, kind="ExternalOutput").ap()
    y_d = nc.dram_tensor("y", [SQ, DM], F32, kind="ExternalOutput").ap()

    with tile.TileContext(nc) as tc, ExitStack() as ctx:
        consts = ctx.enter_context(tc.tile_pool(name="consts", bufs=1))
        bigp = ctx.enter_context(tc.tile_pool(name="bigp", bufs=1))
        stage = ctx.enter_context(tc.tile_pool(name="stage", bufs=3))
        pwork = ctx.enter_context(tc.tile_pool(name="pwork", bufs=5, space="PSUM"))
        pden = ctx.enter_context(tc.tile_pool(name="pden", bufs=2, space="PSUM"))

        # --- tiles ---
        ones128 = consts.tile([P, P], BF16, name="ones128", tag="ones128")
        bl_sb = consts.tile([P, DM], F32, name="bl_sb", tag="bl_sb")
        mb_sb = consts.tile([P, NK], F32, name="mb_sb", tag="mb_sb")
        m_sb = consts.tile([P, ND, DM], BF16, name="m_sb", tag="m_sb")
        wvl_sb = consts.tile([P, ND, DM], BF16, name="wvl_sb", tag="wvl_sb")

        qt_sb = bigp.tile([P, ND, SQ], BF16, name="qt_sb", tag="qt_sb")
        kt_sb = bigp.tile([P, ND, SK], BF16, name="kt_sb", tag="kt_sb")
        vt_sb = bigp.tile([P, ND, SK], BF16, name="vt_sb", tag="vt_sb")
        at_sb = bigp.tile([P, ND, SQ], BF16, name="at_sb", tag="at_sb")
        vl = bigp.tile([P, NK, DM], BF16, name="vl", tag="vl")
        ex = bigp.tile([P, NK, SQ], BF16, name="ex", tag="ex")
        att_n = bigp.tile([P, NK, SQ], BF16, name="att_n", tag="att_n")
        rc = consts.tile([P, SQ], F32, name="rc", tag="rc")
        rcb = consts.tile([P, SQ], BF16, name="rcb", tag="rcb")

        # --- input DMAs, ordered by first use, split per d-block; sync and
        # scalar issue to distinct HWDGE rings (FIFO per ring).
        m_r = m_d.rearrange("(o p) f -> p o f", p=P)
        wvl_r = wvl_d.rearrange("(o p) f -> p o f", p=P)
        qt_r = qt_d.rearrange("(o p) q -> p o q", p=P)
        kt_r = kt_d.rearrange("(o p) q -> p o q", p=P)
        vt_r = vt_d.rearrange("(o p) q -> p o q", p=P)

        # first-needed blocks race down both rings in parallel
        for di in (0, 1):
            nc.sync.dma_start(m_sb[:, di, :], m_r[:, di, :])
            nc.sync.dma_start(qt_sb[:, di, :], qt_r[:, di, :])
        for di in (2, 3):
            nc.scalar.dma_start(m_sb[:, di, :], m_r[:, di, :])
            nc.scalar.dma_start(qt_sb[:, di, :], qt_r[:, di, :])
        for di in (0, 1):
            nc.sync.dma_start(kt_sb[:, di, :], kt_r[:, di, :])
        for di in (2, 3):
            nc.scalar.dma_start(kt_sb[:, di, :], kt_r[:, di, :])
        nc.scalar.dma_start(mb_sb[:], mb_d[:])
        nc.scalar.dma_start(ones128[:], ones_d[:])
        for di in (0, 1):
            nc.sync.dma_start(vt_sb[:, di, :], vt_r[:, di, :])
            nc.sync.dma_start(wvl_sb[:, di, :], wvl_r[:, di, :])
        for di in (2, 3):
            nc.scalar.dma_start(vt_sb[:, di, :], vt_r[:, di, :])
            nc.scalar.dma_start(wvl_sb[:, di, :], wvl_r[:, di, :])
        nc.scalar.dma_start(bl_sb[:], bl_d[:])

        # accumulation order follows DMA arrival (rings fill 0,2 then 1,3)
        DI_ORDER = (0, 2, 1, 3)

        def ps_tile(name):
            return pwork.tile([P, NF], F32, name=name, tag="ps")

        # --- Phase A: AT = M.T @ QT ---
        for dt in range(ND):
            pss = [ps_tile(f"psat_{dt}_{qh}") for qh in range(NH)]
            for j, di in enumerate(DI_ORDER):
                for qh in range(NH):
                    nc.tensor.matmul(
                        pss[qh][:],
                        m_sb[:, di, dt * P:(dt + 1) * P],
                        qt_sb[:, di, qh * NF:(qh + 1) * NF],
                        start=(j == 0),
                        stop=(j == ND - 1),
                    )
            for qh in range(NH):
                nc.scalar.activation(
                    at_sb[:, dt, qh * NF:(qh + 1) * NF], pss[qh][:], AF.Copy
                )

        # --- Phase B: scoresT -> exp -> denominator ---
        pd = [
            pden.tile([P, NF], F32, name=f"pd_{qh}", tag="pden") for qh in range(NH)
        ]
        for kt_i in range(NK):
            pss = [ps_tile(f"pssc_{kt_i}_{qh}") for qh in range(NH)]
            for j, di in enumerate(DI_ORDER):
                for qh in range(NH):
                    nc.tensor.matmul(
                        pss[qh][:],
                        kt_sb[:, di, kt_i * P:(kt_i + 1) * P],
                        at_sb[:, di, qh * NF:(qh + 1) * NF],
                        start=(j == 0),
                        stop=(j == ND - 1),
                    )
            for qh in range(NH):
                qs = slice(qh * NF, (qh + 1) * NF)
                nc.scalar.activation(
                    ex[:, kt_i, qs],
                    pss[qh][:],
                    AF.Exp,
                    bias=mb_sb[:, kt_i:kt_i + 1],
                    scale=SM_SCALE,
                )
                nc.tensor.matmul(
                    pd[qh][:],
                    ones128[:],
                    ex[:, kt_i, qs],
                    start=(kt_i == 0),
                    stop=(kt_i == NK - 1),
                )

        # --- Phase A' (placed here so the PE fills the recip bubble):
        # Vl2 = VT.T @ WVL ---
        for kt_i in range(NK):
            ps = ps_tile(f"psvl_{kt_i}")
            for j, di in enumerate(DI_ORDER):
                nc.tensor.matmul(
                    ps[:],
                    vt_sb[:, di, kt_i * P:(kt_i + 1) * P],
                    wvl_sb[:, di, :],
                    start=(j == 0),
                    stop=(j == ND - 1),
                )
            nc.scalar.activation(vl[:, kt_i, :], ps[:], AF.Copy)

        # --- reciprocal of denominator (replicated rows) + bf16 copy so the
        # normalize muls run in the DVE 16-bit 2x mode ---
        for qh in range(NH):
            nc.vector.reciprocal(rc[:, qh * NF:(qh + 1) * NF], pd[qh][:])
            nc.vector.tensor_copy(
                out=rcb[:, qh * NF:(qh + 1) * NF], in_=rc[:, qh * NF:(qh + 1) * NF]
            )

        # --- normalize att (bf16, feeds Y); att output leaves via a casting
        # gpsimd DMA (bf16 -> f32), no f32 staging pass needed ---
        for kt_i in range(NK):
            for qh in range(NH):
                qs = slice(qh * NF, (qh + 1) * NF)
                nc.vector.tensor_mul(
                    out=att_n[:, kt_i, qs], in0=ex[:, kt_i, qs], in1=rcb[:, qs]
                )
            nc.gpsimd.dma_start(
                att_d[kt_i * P:(kt_i + 1) * P, :], att_n[:, kt_i, :]
            )

        # --- Phase Y: Y[q, :] = sum_kt att_n[kt].T @ Vl2[kt] + bl2 ---
        for qi in range(NQ):
            ps = ps_tile(f"psy_{qi}")
            for kt_i in range(NK):
                nc.tensor.matmul(
                    ps[:],
                    att_n[:, kt_i, qi * P:(qi + 1) * P],
                    vl[:, kt_i, :],
                    start=(kt_i == 0),
                    stop=(kt_i == NK - 1),
                )
            y_sb = stage.tile([P, DM], F32, name=f"y_sb_{qi}", tag="y_sb")
            nc.vector.tensor_add(out=y_sb[:], in0=ps[:], in1=bl_sb[:])
            nc.sync.dma_start(y_d[qi * P:(qi + 1) * P, :], y_sb[:])

    nc.compile()
    return nc


_NC_CACHE = {}


def get_nc():
    if "nc" not in _NC_CACHE:
        _NC_CACHE["nc"] = build_bass()
    return _NC_CACHE["nc"]


def prepare_in_maps(Q, K, V, mask, Wq, bq, Wk, bk, Wv, bv, Wl, bl):
    f = lambda a: np.ascontiguousarray(np.asarray(a, dtype=np.float32))
    Q, K, V = f(Q), f(K), f(V)
    Wq, Wk, Wv, Wl = f(Wq), f(Wk), f(Wv), f(Wl)
    bq, bk, bv, bl = f(bq), f(bk), f(bv), f(bl)
    mask = np.asarray(mask)

    bf = ml_dtypes.bfloat16
    g = lambda a: np.ascontiguousarray(a.astype(bf))

    wls = Wl.reshape(H, DM, DM).sum(axis=0, dtype=np.float64)
    m = (Wq.astype(np.float64) @ Wk.astype(np.float64).T).astype(np.float32)
    wvl = (Wv.astype(np.float64) @ wls).astype(np.float32)
    bl2 = (bv.astype(np.float64) @ wls + bl).astype(np.float32)
    blr2 = np.ascontiguousarray(np.broadcast_to(bl2, (P, DM)))
    wkbq = Wk @ bq  # [512]; u = K @ wkbq is the only surviving bias term

    in_maps = []
    for b in range(B):
        u = K[b] @ wkbq                                   # [1024]
        mb = mask[b, 0].astype(np.float32) * np.float32(-1e9) \
            + np.float32(SM_SCALE) * u
        in_maps.append(
            {
                "qt": g(Q[b].T),
                "kt": g(K[b].T),
                "vt": g(V[b].T),
                "m": g(m),
                "wvl": g(wvl),
                "blr2": blr2,
                "mb": np.ascontiguousarray(mb.reshape(NK, P).T),  # [128, 8]
                "ones": np.ones((P, P), dtype=bf),
            }
        )
    return in_maps


def postprocess(results):
    Y = np.stack([np.asarray(results[b]["y"]) for b in range(B)])
    att = np.stack([np.asarray(results[b]["attT"]).T for b in range(B)])
    att_ws = np.broadcast_to(att[:, None], (B, H, SQ, SK))
    return Y, att_ws


def kernel(Q, K, V, mask, Wq, bq, Wk, bk, Wv, bv, Wl, bl):
    nc = get_nc()
    in_maps = prepare_in_maps(Q, K, V, mask, Wq, bq, Wk, bk, Wv, bv, Wl, bl)
    res = run_bass_kernel_spmd(nc, in_maps, list(range(B)))
    return postprocess(res.results)
